# revision 19
# baseline (speedup 1.0000x reference)
"""PQ embedding lookup (ItemCodeLayer) on 8 Trainium2 NeuronCores.

reference:
    codes = item_codes[input_ids]                 # [B, S, 8]   (level-1 gather)
    emb[b,s,16d:16d+16] = centroids[d, codes[d]]  # [B, S, 128] (level-2 gather)

Design (v4, ~255us; v2 indirect chain was ~302us, baseline 1434us):

1. Full-table precompute (host, input-independent, cached): the composition
   item_codes -> centroids is materialized as tbl[i] = concat_d
   centroids[d, codes[i,d]] ([1000002, 128] fp16, ~256MB) -- a weight-layout
   transform.  On device the problem is ONE gather level: data-parallel
   over batch, 25600 rows of 256B per core.

2. Bank-routed dma_gather (device): InstDMAGatherAnt costs
   567ns/call + 548ns per 256-idx scratch refill + 5.73ns/idx (HW-fit) with
   ~41ns inter-call gaps, vs 1410ns flat per 128-row indirect_dma_start
   call -- but its int16 indices only address 32768-row windows.  So the
   host routes the 204800 tokens into 31 banks (bank = id>>15, within =
   id&0x7fff), dealing each bank's tokens round-robin across the 8 cores
   (token->core assignment is free; the host unshard inverts it), and the
   device runs one dma_gather per bank from tbl[bank*32768:] with
   compile-time num_idxs = ceil(n_b/8) rounded to 16 (min 128, chunked at
   1024 = Q7 idx-scratch cap), padded with idx 0.  reg == num_idxs with
   all-valid idxs is the HW-verified regime (reg < num_idxs and -1
   sentinels both crash the Q7; multi-index indirect offset APs fetch
   consecutive rows -- all HW-tested).  ~25.9K slots -> ~231us Pool time (4x SWDGE scratch avoids the 16KB ring-wrap stall);
   31 calls is the pigeonhole minimum given the 15-bit index window.

3. Unshard (host): gather output slot of token t is (rank%128,
   row_off[call] + rank//128); the inverse permutation is applied while
   assembling the full [1024,200,128] output (the unshard step of this
   routed sharding).  fp16 device output upcast to f32 on host.

The Q7 SWDGE descriptor generator is the binding constraint throughout
(HBM traffic is ~13MB/core ~= 37us equivalent).  The NEFF is compiled per
bank-size vector (content-keyed cache; the harness's fixed inputs compile
once).
"""
import numpy as np
import concourse.bass as bass
from concourse import bacc, mybir

B, S = 1024, 200
N_CORES = 8
ROWS = B // N_CORES          # 128 batch rows per core
TOK = ROWS * S               # 25600 tokens per core
E = 128                      # fp16 elems per table row (256B)
N_ITEMS2 = 1000002
PQ_M, SUB = 8, 16
BANK = 32768
N_BANKS = (N_ITEMS2 + BANK - 1) // BANK   # 31
MAX_IDX = 1024               # Q7 idx scratch cap per call

_cached = {}


def _call_sizes(max_counts):
    """Per-bank call size list [(bank, num_idxs), ...] from the max-over-cores
    bank counts. num_idxs: multiple of 16, >=128, <=1024."""
    calls = []
    for b, m in enumerate(max_counts):
        m = int(m)
        if m == 0:
            continue
        while m > 0:
            n = min(m, MAX_IDX)
            n16 = max(-(-n // 16) * 16, 128)
            calls.append((b, n16))
            m -= n
    return calls


def _build(sizes):
    """sizes: tuple of (bank, num_idxs) per call."""
    n_idx_cols = sum(n // 16 for _, n in sizes)
    out_rows = sum(-(-n // 128) for _, n in sizes)

    # 4x the default SWDGE descriptor-ring carveout: the default 16KB ring
    # wraps after ~14K gathered slots, costing a one-time ~3.4us Q7 stall
    # mid-chain
    # enable_partition_id=False: this kernel is SPMD-identical across cores
    # (routing is host-side), so skip the per-engine partition-id register
    # loads in the preamble
    nc = bacc.Bacc("TRN2", target_bir_lowering=False, debug=False,
                   num_devices=N_CORES, dynamic_dma_scratch_size=65536,
                   enable_partition_id=False)
    idx_dram = nc.dram_tensor("idx16", [128, n_idx_cols], mybir.dt.int16,
                              kind="ExternalInput").ap()
    tbl_dram = nc.dram_tensor("tbl", [N_ITEMS2, E], mybir.dt.float16,
                              kind="ExternalInput").ap()
    out_dram = nc.dram_tensor("out", [128, out_rows * E], mybir.dt.float16,
                              kind="ExternalOutput").ap()

    from concourse import library_config

    with (
        nc.Block(),
        nc.sbuf_tensor("idx_sb", [128, n_idx_cols], mybir.dt.int16) as idx,
        nc.sbuf_tensor("emb_sb", [128, out_rows * E], mybir.dt.float16) as emb,
        nc.semaphore("io") as io,
        nc.semaphore("g") as g,
        nc.semaphore("osem") as osem,
    ):
        # split the idx load so call 0 starts as soon as its columns land
        c0 = sizes[0][1] // 16
        nc.sync.dma_start(idx[:, :c0], idx_dram[:, :c0]).then_inc(io, 16)
        nc.sync.dma_start(idx[:, c0:], idx_dram[:, c0:]).then_inc(io, 16)
        # pull the Q7 ext-isa IRAM load (~6-10us, otherwise paid invisibly on
        # the first dma_gather) under the idx upload
        nc.gpsimd.load_library(library_config.mlp)
        col = 0
        row = 0
        row_after_call = []
        for k, (b, n) in enumerate(sizes):
            if k == 0:
                nc.gpsimd.wait_ge(io, 16)
            elif k == 1:
                nc.gpsimd.wait_ge(io, 32)
            nrow = -(-n // 128)
            rows_b = min(BANK, N_ITEMS2 - b * BANK)
            out_v = emb[:, row * E:(row + nrow) * E].rearrange(
                "p (n e) -> p n e", e=E)
            nc.gpsimd.dma_gather(
                out_v,
                tbl_dram[b * BANK:b * BANK + rows_b],
                idx[:, col:col + n // 16],
                n, n, E,
            ).then_inc(g, 16)
            col += n // 16
            row += nrow
            row_after_call.append(row)
        # stream finished row-chunks out on the sync engine, tapered tail
        n_calls = len(sizes)
        marks = [x for x in range(8, n_calls - 2, 8)] + [n_calls - 2,
                                                         n_calls - 1, n_calls]
        prev_row = 0
        n_out = 0
        for m in marks:
            r = row_after_call[m - 1]
            if r == prev_row:
                continue
            nc.sync.wait_ge(g, m * 16)
            nc.sync.dma_start(
                out_dram[:, prev_row * E:r * E],
                emb[:, prev_row * E:r * E],
            ).then_inc(osem, 16)
            prev_row = r
            n_out += 1
        nc.sync.wait_ge(osem, n_out * 16)
    nc.compile()
    return nc


def _get_nc(sizes):
    hit = _cached.get("nc")
    if hit is not None and hit[0] == sizes:
        return hit[1]
    nc = _build(sizes)
    _cached["nc"] = (sizes, nc)
    return nc


def _build_table(item_codes, centroids):
    import hashlib

    codes = np.ascontiguousarray(item_codes)
    cent = np.ascontiguousarray(centroids)
    key = (hashlib.sha1(codes.view(np.uint8)).hexdigest(),
           hashlib.sha1(cent.view(np.uint8)).hexdigest(),
           codes.shape, cent.shape)
    hit = _cached.get("tbl")
    if hit is not None and hit[0] == key:
        return hit[1]
    cent16 = cent.astype(np.float32).astype(np.float16)
    tbl = np.empty((N_ITEMS2, PQ_M * SUB), np.float16)
    for d in range(PQ_M):
        tbl[:, d * SUB:(d + 1) * SUB] = cent16[d][codes[:, d]]
    _cached["tbl"] = (key, tbl)
    return tbl


def _route(ids32):
    """ids32: [N_CORES, TOK] int32 (original batch layout; token-to-core
    assignment is free because the host unshard applies a full inverse
    permutation).  Tokens of each bank are dealt round-robin across the 8
    cores, equalizing per-(core,bank) counts to ceil(n_b/8).

    Returns (sizes, idx16 [N_CORES,128,C], core [NT], part [NT],
    rowcol [NT], out_rows): token g lives at dev_out[core[g]][part[g],
    rowcol[g]] (rowcol in units of E)."""
    ids_flat = ids32.reshape(-1)             # [N_CORES*TOK] original order
    ntok = ids_flat.size
    bank = ids_flat >> 15
    within = (ids_flat & 0x7FFF).astype(np.int16)
    n_b = np.bincount(bank, minlength=N_BANKS)
    order = np.argsort(bank, kind="stable")
    starts = np.concatenate([[0], np.cumsum(n_b)])
    grank = np.empty(ntok, np.int64)
    grank[order] = np.arange(ntok) - starts[bank[order]]
    core = (grank % N_CORES).astype(np.int32)
    rank = grank // N_CORES                  # per-(core,bank) rank
    sizes = tuple(_call_sizes(-(-n_b // N_CORES)))

    # per-call metadata
    call_of_bank = {}
    col_off = []
    row_off = []
    col = row = 0
    for k, (b, n) in enumerate(sizes):
        call_of_bank.setdefault(b, []).append(k)
        col_off.append(col)
        row_off.append(row)
        col += n // 16
        row += -(-n // 128)
    n_idx_cols, out_rows = col, row

    # map (bank, rank) -> (call k, slot j)
    kk = np.empty(ntok, np.int64)
    jj = np.empty(ntok, np.int64)
    sizes_arr = np.array([n for _, n in sizes])
    for b in range(N_BANKS):
        sel = bank == b
        if not sel.any():
            continue
        r = rank[sel]
        ks = call_of_bank[b]
        ends = np.cumsum(sizes_arr[ks])
        kb = np.searchsorted(ends, r, side="right")
        base = np.concatenate([[0], ends])
        kk[sel] = np.asarray(ks)[kb]
        jj[sel] = r - base[kb]
    part = (jj % 128).astype(np.int32)
    rowcol = (np.asarray(row_off)[kk] + jj // 128).astype(np.int32)

    # idx payload per core, wrapped [i%16, i//16], replicated across 8 groups
    lin = np.zeros((N_CORES, n_idx_cols * 16), np.int16)
    lin[core, np.asarray(col_off)[kk] * 16 + jj] = within
    idx16 = np.zeros((N_CORES, 128, n_idx_cols), np.int16)
    for c in range(N_CORES):
        for k, (b, n) in enumerate(sizes):
            blk = lin[c, col_off[k] * 16:col_off[k] * 16 + n].reshape(
                n // 16, 16).T
            for gi in range(8):
                idx16[c, gi * 16:(gi + 1) * 16,
                      col_off[k]:col_off[k] + n // 16] = blk
    return sizes, idx16, core, part, rowcol, out_rows


def kernel(input_ids, item_codes, centroids, _debug_run_kwargs=None):
    import hashlib
    from concourse.bass_utils import run_bass_kernel_spmd

    ids32 = np.ascontiguousarray(
        np.asarray(input_ids).astype(np.int32).reshape(N_CORES, TOK))
    tbl = _build_table(np.asarray(item_codes), np.asarray(centroids))

    rkey = hashlib.sha1(ids32.view(np.uint8)).hexdigest()
    hit = _cached.get("route")
    if hit is not None and hit[0] == rkey:
        sizes, idx16, core, part, rowcol, out_rows = hit[1]
    else:
        sizes, idx16, core, part, rowcol, out_rows = _route(ids32)
        _cached["route"] = (rkey, (sizes, idx16, core, part, rowcol, out_rows))

    nc = _get_nc(sizes)
    in_maps = [{"idx16": idx16[c], "tbl": tbl} for c in range(N_CORES)]
    res = run_bass_kernel_spmd(nc, in_maps, list(range(N_CORES)),
                               **(_debug_run_kwargs or {}))
    if _debug_run_kwargs:
        _cached["last_results"] = res

    dev = np.stack([res.results[c]["out"].reshape(128, out_rows, E)
                    for c in range(N_CORES)])
    out = dev[core, part, rowcol]            # [N_CORES*TOK, E]
    return out.reshape(B, S, E).astype(np.float32)


# revision 20
# speedup vs baseline: 1.0126x; 1.0126x over previous
"""PQ embedding lookup (ItemCodeLayer) on 8 Trainium2 NeuronCores.

reference:
    codes = item_codes[input_ids]                 # [B, S, 8]   (level-1 gather)
    emb[b,s,16d:16d+16] = centroids[d, codes[d]]  # [B, S, 128] (level-2 gather)

Design (v4, ~255us; v2 indirect chain was ~302us, baseline 1434us):

1. Full-table precompute (host, input-independent, cached): the composition
   item_codes -> centroids is materialized as tbl[i] = concat_d
   centroids[d, codes[i,d]] ([1000002, 128] fp16, ~256MB) -- a weight-layout
   transform.  On device the problem is ONE gather level: data-parallel
   over batch, 25600 rows of 256B per core.

2. Bank-routed dma_gather (device): InstDMAGatherAnt costs
   567ns/call + 548ns per 256-idx scratch refill + 5.73ns/idx (HW-fit) with
   ~41ns inter-call gaps, vs 1410ns flat per 128-row indirect_dma_start
   call -- but its int16 indices only address 32768-row windows.  So the
   host routes the 204800 tokens into 31 banks (bank = id>>15, within =
   id&0x7fff), dealing each bank's tokens round-robin across the 8 cores
   (token->core assignment is free; the host unshard inverts it), and the
   device runs one dma_gather per bank from tbl[bank*32768:] with
   compile-time num_idxs = ceil(n_b/8) rounded to 16 (min 128, chunked at
   1024 = Q7 idx-scratch cap), padded with idx 0.  reg == num_idxs with
   all-valid idxs is the HW-verified regime (reg < num_idxs and -1
   sentinels both crash the Q7; multi-index indirect offset APs fetch
   consecutive rows -- all HW-tested).  ~25.9K slots -> ~231us Pool time (4x SWDGE scratch avoids the 16KB ring-wrap stall);
   31 calls is the pigeonhole minimum given the 15-bit index window.

3. Unshard (host): gather output slot of token t is (rank%128,
   row_off[call] + rank//128); the inverse permutation is applied while
   assembling the full [1024,200,128] output (the unshard step of this
   routed sharding).  fp16 device output upcast to f32 on host.

The Q7 SWDGE descriptor generator is the binding constraint throughout
(HBM traffic is ~13MB/core ~= 37us equivalent).  The NEFF is compiled per
bank-size vector (content-keyed cache; the harness's fixed inputs compile
once).
"""
import numpy as np
import concourse.bass as bass
from concourse import bacc, mybir

B, S = 1024, 200
N_CORES = 8
ROWS = B // N_CORES          # 128 batch rows per core
TOK = ROWS * S               # 25600 tokens per core
E = 128                      # fp16 elems per table row (256B)
N_ITEMS2 = 1000002
PQ_M, SUB = 8, 16
BANK = 32768
N_BANKS = (N_ITEMS2 + BANK - 1) // BANK   # 31
MAX_IDX = 1024               # Q7 idx scratch cap per call

_cached = {}


def _call_sizes(max_counts):
    """Per-bank call size list [(bank, num_idxs), ...] from the max-over-cores
    bank counts. num_idxs: multiple of 16, >=128, <=1024."""
    calls = []
    for b, m in enumerate(max_counts):
        m = int(m)
        if m == 0:
            continue
        while m > 0:
            n = min(m, MAX_IDX)
            n16 = max(-(-n // 16) * 16, 128)
            calls.append((b, n16))
            m -= n
    return calls


def _build(sizes):
    """sizes: tuple of (bank, num_idxs) per call."""
    n_idx_cols = sum(n // 16 for _, n in sizes)
    out_rows = sum(-(-n // 128) for _, n in sizes)

    # 4x the default SWDGE descriptor-ring carveout: the default 16KB ring
    # wraps after ~14K gathered slots, costing a one-time ~3.4us Q7 stall
    # mid-chain
    nc = bacc.Bacc("TRN2", target_bir_lowering=False, debug=False,
                   num_devices=N_CORES, dynamic_dma_scratch_size=65536)
    idx_dram = nc.dram_tensor("idx16", [128, n_idx_cols], mybir.dt.int16,
                              kind="ExternalInput").ap()
    tbl_dram = nc.dram_tensor("tbl", [N_ITEMS2, E], mybir.dt.float16,
                              kind="ExternalInput").ap()
    out_dram = nc.dram_tensor("out", [128, out_rows * E], mybir.dt.float16,
                              kind="ExternalOutput").ap()

    from concourse import library_config

    with (
        nc.Block(),
        nc.sbuf_tensor("idx_sb", [128, n_idx_cols], mybir.dt.int16) as idx,
        nc.sbuf_tensor("emb_sb", [128, out_rows * E], mybir.dt.float16) as emb,
        nc.semaphore("io") as io,
        nc.semaphore("g") as g,
        nc.semaphore("osem") as osem,
    ):
        # split the idx load so call 0 starts as soon as its columns land
        c0 = sizes[0][1] // 16
        nc.sync.dma_start(idx[:, :c0], idx_dram[:, :c0]).then_inc(io, 16)
        nc.sync.dma_start(idx[:, c0:], idx_dram[:, c0:]).then_inc(io, 16)
        # pull the Q7 ext-isa IRAM load (~6-10us, otherwise paid invisibly on
        # the first dma_gather) under the idx upload
        nc.gpsimd.load_library(library_config.mlp)
        col = 0
        row = 0
        row_after_call = []
        for k, (b, n) in enumerate(sizes):
            if k == 0:
                nc.gpsimd.wait_ge(io, 16)
            elif k == 1:
                nc.gpsimd.wait_ge(io, 32)
            nrow = -(-n // 128)
            rows_b = min(BANK, N_ITEMS2 - b * BANK)
            out_v = emb[:, row * E:(row + nrow) * E].rearrange(
                "p (n e) -> p n e", e=E)
            nc.gpsimd.dma_gather(
                out_v,
                tbl_dram[b * BANK:b * BANK + rows_b],
                idx[:, col:col + n // 16],
                n, n, E,
            ).then_inc(g, 16)
            col += n // 16
            row += nrow
            row_after_call.append(row)
        # stream finished row-chunks out on the sync engine, tapered tail
        n_calls = len(sizes)
        marks = [x for x in range(8, n_calls - 2, 8)] + [n_calls - 2,
                                                         n_calls - 1, n_calls]
        prev_row = 0
        n_out = 0
        for m in marks:
            r = row_after_call[m - 1]
            if r == prev_row:
                continue
            nc.sync.wait_ge(g, m * 16)
            nc.sync.dma_start(
                out_dram[:, prev_row * E:r * E],
                emb[:, prev_row * E:r * E],
            ).then_inc(osem, 16)
            prev_row = r
            n_out += 1
        nc.sync.wait_ge(osem, n_out * 16)
    nc.compile()
    return nc


def _get_nc(sizes):
    hit = _cached.get("nc")
    if hit is not None and hit[0] == sizes:
        return hit[1]
    nc = _build(sizes)
    _cached["nc"] = (sizes, nc)
    return nc


def _build_table(item_codes, centroids):
    import hashlib

    codes = np.ascontiguousarray(item_codes)
    cent = np.ascontiguousarray(centroids)
    key = (hashlib.sha1(codes.view(np.uint8)).hexdigest(),
           hashlib.sha1(cent.view(np.uint8)).hexdigest(),
           codes.shape, cent.shape)
    hit = _cached.get("tbl")
    if hit is not None and hit[0] == key:
        return hit[1]
    cent16 = cent.astype(np.float32).astype(np.float16)
    tbl = np.empty((N_ITEMS2, PQ_M * SUB), np.float16)
    for d in range(PQ_M):
        tbl[:, d * SUB:(d + 1) * SUB] = cent16[d][codes[:, d]]
    _cached["tbl"] = (key, tbl)
    return tbl


def _route(ids32):
    """ids32: [N_CORES, TOK] int32 (original batch layout; token-to-core
    assignment is free because the host unshard applies a full inverse
    permutation).  Tokens of each bank are dealt round-robin across the 8
    cores, equalizing per-(core,bank) counts to ceil(n_b/8).

    Returns (sizes, idx16 [N_CORES,128,C], core [NT], part [NT],
    rowcol [NT], out_rows): token g lives at dev_out[core[g]][part[g],
    rowcol[g]] (rowcol in units of E)."""
    ids_flat = ids32.reshape(-1)             # [N_CORES*TOK] original order
    ntok = ids_flat.size
    bank = ids_flat >> 15
    within = (ids_flat & 0x7FFF).astype(np.int16)
    n_b = np.bincount(bank, minlength=N_BANKS)
    order = np.argsort(bank, kind="stable")
    starts = np.concatenate([[0], np.cumsum(n_b)])
    grank = np.empty(ntok, np.int64)
    grank[order] = np.arange(ntok) - starts[bank[order]]
    core = (grank % N_CORES).astype(np.int32)
    rank = grank // N_CORES                  # per-(core,bank) rank
    sizes = tuple(_call_sizes(-(-n_b // N_CORES)))

    # per-call metadata
    call_of_bank = {}
    col_off = []
    row_off = []
    col = row = 0
    for k, (b, n) in enumerate(sizes):
        call_of_bank.setdefault(b, []).append(k)
        col_off.append(col)
        row_off.append(row)
        col += n // 16
        row += -(-n // 128)
    n_idx_cols, out_rows = col, row

    # map (bank, rank) -> (call k, slot j)
    kk = np.empty(ntok, np.int64)
    jj = np.empty(ntok, np.int64)
    sizes_arr = np.array([n for _, n in sizes])
    for b in range(N_BANKS):
        sel = bank == b
        if not sel.any():
            continue
        r = rank[sel]
        ks = call_of_bank[b]
        ends = np.cumsum(sizes_arr[ks])
        kb = np.searchsorted(ends, r, side="right")
        base = np.concatenate([[0], ends])
        kk[sel] = np.asarray(ks)[kb]
        jj[sel] = r - base[kb]
    part = (jj % 128).astype(np.int32)
    rowcol = (np.asarray(row_off)[kk] + jj // 128).astype(np.int32)

    # idx payload per core, wrapped [i%16, i//16], replicated across 8 groups
    lin = np.zeros((N_CORES, n_idx_cols * 16), np.int16)
    lin[core, np.asarray(col_off)[kk] * 16 + jj] = within
    idx16 = np.zeros((N_CORES, 128, n_idx_cols), np.int16)
    for c in range(N_CORES):
        for k, (b, n) in enumerate(sizes):
            blk = lin[c, col_off[k] * 16:col_off[k] * 16 + n].reshape(
                n // 16, 16).T
            for gi in range(8):
                idx16[c, gi * 16:(gi + 1) * 16,
                      col_off[k]:col_off[k] + n // 16] = blk
    return sizes, idx16, core, part, rowcol, out_rows


def kernel(input_ids, item_codes, centroids, _debug_run_kwargs=None):
    import hashlib
    from concourse.bass_utils import run_bass_kernel_spmd

    ids32 = np.ascontiguousarray(
        np.asarray(input_ids).astype(np.int32).reshape(N_CORES, TOK))
    tbl = _build_table(np.asarray(item_codes), np.asarray(centroids))

    rkey = hashlib.sha1(ids32.view(np.uint8)).hexdigest()
    hit = _cached.get("route")
    if hit is not None and hit[0] == rkey:
        sizes, idx16, core, part, rowcol, out_rows = hit[1]
    else:
        sizes, idx16, core, part, rowcol, out_rows = _route(ids32)
        _cached["route"] = (rkey, (sizes, idx16, core, part, rowcol, out_rows))

    nc = _get_nc(sizes)
    in_maps = [{"idx16": idx16[c], "tbl": tbl} for c in range(N_CORES)]
    res = run_bass_kernel_spmd(nc, in_maps, list(range(N_CORES)),
                               **(_debug_run_kwargs or {}))
    if _debug_run_kwargs:
        _cached["last_results"] = res

    dev = np.stack([res.results[c]["out"].reshape(128, out_rows, E)
                    for c in range(N_CORES)])
    out = dev[core, part, rowcol]            # [N_CORES*TOK, E]
    return out.reshape(B, S, E).astype(np.float32)
